# revision 2
# baseline (speedup 1.0000x reference)
"""DBRX-style MoE (E=16, top-4, C=2048, H=3584, N=1024 tokens) on 8 TRN2 cores.

Strategy (expert-parallel with H-split load balancing):
  - Host: gating in fp64 (logits -> top-4 -> softmax weights). fp64 makes the
    selected expert SET maximally robust against fp rounding.
  - Work unit = (expert, H-half): each expert's ffn_hidden axis is split into
    two halves of 14 h-chunks. 32 units are rank-blocked by token count into
    4 slots x 8 cores, so each core runs 4 units. Splitting H decouples the
    two halves across cores, shrinking the SPMD padding tax from
    max(c)+c_(9) tokens/core to (c_(1)+c_(5)+c_(9)+c_(13))/2: the per-slot
    cap is the max count within its rank block. Weight DMA is unchanged (the
    H-split partitions W_up/W_gate/W_down exactly once across cores).
  - Device (per core, per unit): uT/gT = Wup/Wg @ xT (PSUM-accumulated over
    C chunks, f16 matmuls), hT = silu(gT) * uT * gate_weight, then
    yT = Wdown_half @ hT accumulated over the unit's 14 h-chunks. Partial
    outputs are written f16 (halves write traffic; partials are summed on
    the host in fp32, well within the error budget).
  - Host: scatter-add each unit's yT columns back to its token rows.

Warm-up: ~40 dummy matmuls on a zeroed tile run while the first input DMAs
stream, so the PE HAM clock-gate is already at 8/8 when real work starts.

Padding slots have gate weight 0 and their yT columns are never read back.
"""

import math

import numpy as np

E, TOPK = 16, 4
C, H = 2048, 3584
B, T = 2, 512
N = B * T
N_CORES = 8
C_CHUNKS = C // 128  # 16
H_CHUNKS = H // 128  # 28
HALVES = 2
HCH = H_CHUNKS // HALVES  # 14 h-chunks per unit
N_SLOTS = E * HALVES // N_CORES  # 4 units per core
WARMUP_MMS = 40

_NC_CACHE: dict[tuple, object] = {}


def _pad4(v: int) -> int:
    return max(64, int(math.ceil(v / 4)) * 4)


def _build_nc(caps: tuple):
    import concourse.bacc as bacc
    import concourse.mybir as mybir
    import concourse.tile as tile

    f32 = mybir.dt.float32
    f16 = mybir.dt.float16

    nc = bacc.Bacc("TRN2", target_bir_lowering=False, debug=False)
    xgs = [
        nc.dram_tensor(f"xg{j}", [128, C_CHUNKS * caps[j]], f16, kind="ExternalInput")
        for j in range(N_SLOTS)
    ]
    wbs = [
        nc.dram_tensor(f"wb{j}", [128, caps[j]], f32, kind="ExternalInput")
        for j in range(N_SLOTS)
    ]
    wug = nc.dram_tensor(
        "wug", [N_SLOTS, HCH, 2, 128, C_CHUNKS * 128], f16, kind="ExternalInput"
    )
    wd = nc.dram_tensor(
        "wd", [N_SLOTS, C_CHUNKS, 128, HCH * 128], f16, kind="ExternalInput"
    )
    yts = [
        nc.dram_tensor(f"yt{j}", [C_CHUNKS, 128, caps[j]], f16, kind="ExternalOutput")
        for j in range(N_SLOTS)
    ]

    with tile.TileContext(nc) as tc:
        with (
            tc.tile_pool(name="warm", bufs=1) as wmp,
            tc.tile_pool(name="xp", bufs=2) as xp,
            tc.tile_pool(name="wp", bufs=8) as wp,
            tc.tile_pool(name="hp", bufs=2) as hp,
            tc.tile_pool(name="wdp", bufs=4) as wdp,
            tc.tile_pool(name="sp", bufs=3) as sp,
            tc.tile_pool(name="psw", bufs=1, space="PSUM") as psw,
            tc.tile_pool(name="psu", bufs=2, space="PSUM") as psu,
            tc.tile_pool(name="psg", bufs=2, space="PSUM") as psg,
            tc.tile_pool(name="psy", bufs=2, space="PSUM") as psy,
        ):
            # HAM warm-up: keep the PE busy during the initial input DMAs so
            # the clock gate reaches 8/8 before the first real matmul.
            if WARMUP_MMS:
                wmt = wmp.tile([128, 128], f16, tag="warm")
                nc.vector.memset(wmt[:], 0.0)
                wps = psw.tile([128, 128], f32, tag="warmps")
                for i in range(WARMUP_MMS):
                    nc.tensor.matmul(
                        wps[:], wmt[:], wmt[:],
                        start=(i == 0), stop=(i == WARMUP_MMS - 1),
                    )

            for u in range(N_SLOTS):
                cap = caps[u]
                xt = xp.tile([128, C_CHUNKS * cap], f16, tag="xg")
                # grouped chunk DMAs: the first c-chunks land early so the
                # first accumulation doesn't wait on the whole gather
                for c0, g in ((0, 1), (1, 3), (4, 6), (10, 6)):
                    nc.sync.dma_start(
                        xt[:, c0 * cap : (c0 + g) * cap],
                        xgs[u].ap()[:, c0 * cap : (c0 + g) * cap],
                    )
                wbt = xp.tile([128, cap], f32, tag="wb")
                nc.sync.dma_start(wbt[:], wbs[u].ap())
                ht = hp.tile([128, HCH * cap], f16, tag="ht")

                for h in range(HCH):
                    wu = wp.tile([128, C_CHUNKS * 128], f16, tag="wug")
                    nc.sync.dma_start(wu[:], wug.ap()[u, h, 0])
                    wg = wp.tile([128, C_CHUNKS * 128], f16, tag="wug")
                    nc.sync.dma_start(wg[:], wug.ap()[u, h, 1])
                    ups = psu.tile([128, cap], f32, tag="u")
                    gps = psg.tile([128, cap], f32, tag="g")
                    for c in range(C_CHUNKS):
                        nc.tensor.matmul(
                            ups[:],
                            wu[:, c * 128 : (c + 1) * 128],
                            xt[:, c * cap : (c + 1) * cap],
                            start=(c == 0),
                            stop=(c == C_CHUNKS - 1),
                        )
                    for c in range(C_CHUNKS):
                        nc.tensor.matmul(
                            gps[:],
                            wg[:, c * 128 : (c + 1) * 128],
                            xt[:, c * cap : (c + 1) * cap],
                            start=(c == 0),
                            stop=(c == C_CHUNKS - 1),
                        )
                    sg = sp.tile([128, cap], f32, tag="sg")
                    nc.scalar.activation(
                        sg[:], gps[:], mybir.ActivationFunctionType.Silu
                    )
                    uw = sp.tile([128, cap], f32, tag="uw")
                    nc.vector.tensor_mul(uw[:], ups[:], wbt[:])
                    nc.vector.tensor_mul(
                        ht[:, h * cap : (h + 1) * cap], sg[:], uw[:]
                    )

                for ct in range(C_CHUNKS):
                    wdt = wdp.tile([128, HCH * 128], f16, tag="wd")
                    nc.sync.dma_start(wdt[:], wd.ap()[u, ct])
                    yps = psy.tile([128, cap], f32, tag="y")
                    for h in range(HCH):
                        nc.tensor.matmul(
                            yps[:],
                            wdt[:, h * 128 : (h + 1) * 128],
                            ht[:, h * cap : (h + 1) * cap],
                            start=(h == 0),
                            stop=(h == HCH - 1),
                        )
                    yo = sp.tile([128, cap], f16, tag="yo")
                    nc.vector.tensor_copy(yo[:], yps[:])
                    nc.sync.dma_start(yts[u].ap()[ct], yo[:])
    nc.compile()
    return nc


def _get_nc(caps: tuple):
    if caps not in _NC_CACHE:
        _NC_CACHE[caps] = _build_nc(caps)
    return _NC_CACHE[caps]


def _route(xf: np.ndarray, gate_inp: np.ndarray):
    """Host gating in fp64: per-expert token index lists + combine weights."""
    logits = xf.astype(np.float64) @ gate_inp.astype(np.float64).T  # [N, E]
    # top-4 (descending); fp64 makes ordering robust vs the fp32 reference
    topi = np.argsort(-logits, axis=1, kind="stable")[:, :TOPK]  # [N, K]
    topv = np.take_along_axis(logits, topi, axis=1)
    w = np.exp(topv - topv[:, :1])
    w /= w.sum(axis=1, keepdims=True)  # [N, K] fp64 softmax
    idxs, wts = [], []
    for e in range(E):
        sel = topi == e  # [N, K]
        rows = np.nonzero(sel.any(axis=1))[0]
        k_of_row = np.argmax(sel[rows], axis=1)  # which top-k slot holds e
        idxs.append(rows.astype(np.int64))
        wts.append(w[rows, k_of_row])
    return idxs, wts


def _prepare(x, W_up, W_gate, W_down, gate_inp):
    xf = np.ascontiguousarray(np.asarray(x, dtype=np.float32)).reshape(N, C)
    W_up = np.asarray(W_up, dtype=np.float32)
    W_gate = np.asarray(W_gate, dtype=np.float32)
    W_down = np.asarray(W_down, dtype=np.float32)
    gate_inp = np.asarray(gate_inp, dtype=np.float32)

    idxs, wts = _route(xf, gate_inp)
    counts = [len(i) for i in idxs]

    # units: (expert, half), sorted desc by token count; rank blocks of 8
    units = sorted(
        ((e, hf) for e in range(E) for hf in range(HALVES)),
        key=lambda u: -counts[u[0]],
    )
    assign = [[units[j * N_CORES + core] for j in range(N_SLOTS)]
              for core in range(N_CORES)]
    caps = tuple(
        _pad4(max(counts[units[j * N_CORES + k][0]] for k in range(N_CORES)))
        for j in range(N_SLOTS)
    )

    # per-expert cached prep: gathered x (f16) and transposed weight forms
    xg_e = {}
    upt_e, gpt_e, wdt_e = {}, {}, {}
    for e in range(E):
        if counts[e]:
            xg_e[e] = xf[idxs[e]].astype(np.float16)
        # [h_chunk, q(c_in), c_chunk, h_col] -> [28, 128, 16*128]
        upt_e[e] = np.ascontiguousarray(
            W_up[e].reshape(H_CHUNKS, 128, C_CHUNKS, 128).transpose(0, 3, 2, 1)
        ).reshape(H_CHUNKS, 128, C_CHUNKS * 128).astype(np.float16)
        gpt_e[e] = np.ascontiguousarray(
            W_gate[e].reshape(H_CHUNKS, 128, C_CHUNKS, 128).transpose(0, 3, 2, 1)
        ).reshape(H_CHUNKS, 128, C_CHUNKS * 128).astype(np.float16)
        # [c_tile, q(h_in), h_chunk, c_col] -> [16, 128, 28, 128]
        wdt_e[e] = np.ascontiguousarray(
            W_down[e].reshape(C_CHUNKS, 128, H_CHUNKS, 128).transpose(0, 3, 2, 1)
        ).astype(np.float16)

    in_maps = []
    for core in range(N_CORES):
        wug = np.empty((N_SLOTS, HCH, 2, 128, C_CHUNKS * 128), np.float16)
        wd = np.empty((N_SLOTS, C_CHUNKS, 128, HCH * 128), np.float16)
        im = {"wug": wug, "wd": wd}
        for j in range(N_SLOTS):
            cap = caps[j]
            e, hf = assign[core][j]
            h0 = hf * HCH
            idx, wvec = idxs[e], wts[e]
            cnt = len(idx)
            xge = np.zeros((cap, C), np.float16)
            if cnt:
                xge[:cnt] = xg_e[e]
            # [q, c_chunk, t] <- xge[t, c_chunk*128+q]
            im[f"xg{j}"] = np.ascontiguousarray(
                xge.reshape(cap, C_CHUNKS, 128).transpose(2, 1, 0)
            ).reshape(128, C_CHUNKS * cap)
            wb = np.zeros((128, cap), np.float32)
            wb[:, :cnt] = np.float32(wvec)[None, :]
            im[f"wb{j}"] = wb
            wug[j, :, 0] = upt_e[e][h0 : h0 + HCH]
            wug[j, :, 1] = gpt_e[e][h0 : h0 + HCH]
            wd[j] = wdt_e[e][:, :, h0 : h0 + HCH, :].reshape(
                C_CHUNKS, 128, HCH * 128
            )
        in_maps.append(im)
    return in_maps, caps, assign, idxs


def _combine(results, caps, assign, idxs):
    y = np.zeros((N, C), np.float32)
    for core in range(N_CORES):
        for j in range(N_SLOTS):
            e, _hf = assign[core][j]
            idx = idxs[e]
            cnt = len(idx)
            if not cnt:
                continue
            ytf = results[core][f"yt{j}"].reshape(C, caps[j])
            y[idx] += ytf[:, :cnt].T.astype(np.float32)
    return y.reshape(B, T, C)


def kernel(x, W_up, W_gate, W_down, gate_inp):
    from concourse import bass_utils

    in_maps, caps, assign, idxs = _prepare(x, W_up, W_gate, W_down, gate_inp)
    nc = _get_nc(caps)
    res = bass_utils.run_bass_kernel_spmd(nc, in_maps, core_ids=list(range(N_CORES)))
    kernel.last_result = res
    return _combine(res.results, caps, assign, idxs)


# revision 3
# speedup vs baseline: 1.1643x; 1.1643x over previous
"""DBRX-style MoE (E=16, top-4, C=2048, H=3584, N=1024 tokens) on 8 TRN2 cores.

Strategy (expert-parallel with H-split load balancing):
  - Host: gating in fp64 (logits -> top-4 -> softmax weights). fp64 makes the
    selected expert SET maximally robust against fp rounding.
  - Work unit = (expert, H-half): each expert's ffn_hidden axis is split into
    two halves of 14 h-chunks. 32 units are rank-blocked by token count into
    4 slots x 8 cores, so each core runs 4 units. Splitting H decouples the
    two halves across cores, shrinking the SPMD padding tax from
    max(c)+c_(9) tokens/core to (c_(1)+c_(5)+c_(9)+c_(13))/2: the per-slot
    cap is the max count within its rank block. Weight DMA is unchanged (the
    H-split partitions W_up/W_gate/W_down exactly once across cores).
  - Device (per core, per unit): uT/gT = Wup/Wg @ xT (PSUM-accumulated over
    C chunks, f16 matmuls), hT = silu(gT) * uT * gate_weight, then
    yT = Wdown_half @ hT accumulated over the unit's 14 h-chunks. Partial
    outputs are staged in SBUF and written f16 (halves write traffic;
    partials are summed on the host in fp32, well within the error budget).
  - Host: scatter-add each unit's yT columns back to its token rows.

DMA layouts keep per-row transfers fat (>= 7KB for the weight streams): the
DMA engines process a roughly fixed row rate, so thin rows starve the PE.
W_up/W_gate pair into one 8KB-row transfer per h-chunk; W_down c-tiles pair
into 7KB-row transfers; y is staged and written as two 4.7KB-row transfers.

Padding slots have gate weight 0 and their yT columns are never read back.
"""

import math

import numpy as np

E, TOPK = 16, 4
C, H = 2048, 3584
B, T = 2, 512
N = B * T
N_CORES = 8
C_CHUNKS = C // 128  # 16
H_CHUNKS = H // 128  # 28
HALVES = 2
HCH = H_CHUNKS // HALVES  # 14 h-chunks per unit
N_SLOTS = E * HALVES // N_CORES  # 4 units per core
CT_PAIRS = C_CHUNKS // 2  # 8 W_down c-tile pairs

_NC_CACHE: dict[tuple, object] = {}


def _pad4(v: int) -> int:
    return max(64, int(math.ceil(v / 4)) * 4)


def _build_nc(caps: tuple):
    import concourse.bacc as bacc
    import concourse.mybir as mybir
    import concourse.tile as tile

    f32 = mybir.dt.float32
    f16 = mybir.dt.float16

    nc = bacc.Bacc("TRN2", target_bir_lowering=False, debug=False)
    xgs = [
        nc.dram_tensor(f"xg{j}", [128, C_CHUNKS * caps[j]], f16, kind="ExternalInput")
        for j in range(N_SLOTS)
    ]
    wbs = [
        nc.dram_tensor(f"wb{j}", [128, caps[j]], f32, kind="ExternalInput")
        for j in range(N_SLOTS)
    ]
    # u and g weights paired per h-chunk: 8KB DMA rows
    wug = nc.dram_tensor(
        "wug", [N_SLOTS, HCH, 128, 2 * C_CHUNKS * 128], f16, kind="ExternalInput"
    )
    # W_down c-tiles paired: 7KB DMA rows
    wd = nc.dram_tensor(
        "wd", [N_SLOTS, CT_PAIRS, 128, 2 * HCH * 128], f16, kind="ExternalInput"
    )
    yts = [
        nc.dram_tensor(f"yt{j}", [128, C_CHUNKS * caps[j]], f16, kind="ExternalOutput")
        for j in range(N_SLOTS)
    ]

    with tile.TileContext(nc) as tc:
        with (
            tc.tile_pool(name="xp", bufs=2) as xp,
            tc.tile_pool(name="wp", bufs=6) as wp,
            tc.tile_pool(name="hp", bufs=2) as hp,
            tc.tile_pool(name="wdp", bufs=5) as wdp,
            tc.tile_pool(name="ysp", bufs=2) as ysp,
            tc.tile_pool(name="sp", bufs=3) as sp,
            tc.tile_pool(name="psu", bufs=2, space="PSUM") as psu,
            tc.tile_pool(name="psg", bufs=2, space="PSUM") as psg,
            tc.tile_pool(name="psy", bufs=2, space="PSUM") as psy,
        ):
            for u in range(N_SLOTS):
                cap = caps[u]
                # first unit: weight DMA first so the first matmul starts ASAP
                wt0 = None
                if u == 0:
                    wt0 = wp.tile([128, 2 * C_CHUNKS * 128], f16, tag="wug")
                    nc.sync.dma_start(wt0[:], wug.ap()[u, 0])
                xt = xp.tile([128, C_CHUNKS * cap], f16, tag="xg")
                # grouped chunk DMAs: the first c-chunks land early so the
                # first accumulation doesn't wait on the whole gather
                for c0, g in ((0, 1), (1, 3), (4, 6), (10, 6)):
                    nc.sync.dma_start(
                        xt[:, c0 * cap : (c0 + g) * cap],
                        xgs[u].ap()[:, c0 * cap : (c0 + g) * cap],
                    )
                wbt = xp.tile([128, cap], f32, tag="wb")
                nc.sync.dma_start(wbt[:], wbs[u].ap())
                ht = hp.tile([128, HCH * cap], f16, tag="ht")

                for h in range(HCH):
                    if h == 0 and wt0 is not None:
                        wt = wt0
                    else:
                        wt = wp.tile([128, 2 * C_CHUNKS * 128], f16, tag="wug")
                        nc.sync.dma_start(wt[:], wug.ap()[u, h])
                    goff = C_CHUNKS * 128
                    ups = psu.tile([128, cap], f32, tag="u")
                    gps = psg.tile([128, cap], f32, tag="g")
                    for c in range(C_CHUNKS):
                        nc.tensor.matmul(
                            ups[:],
                            wt[:, c * 128 : (c + 1) * 128],
                            xt[:, c * cap : (c + 1) * cap],
                            start=(c == 0),
                            stop=(c == C_CHUNKS - 1),
                        )
                    for c in range(C_CHUNKS):
                        nc.tensor.matmul(
                            gps[:],
                            wt[:, goff + c * 128 : goff + (c + 1) * 128],
                            xt[:, c * cap : (c + 1) * cap],
                            start=(c == 0),
                            stop=(c == C_CHUNKS - 1),
                        )
                    sg = sp.tile([128, cap], f32, tag="sg")
                    nc.scalar.activation(
                        sg[:], gps[:], mybir.ActivationFunctionType.Silu
                    )
                    uw = sp.tile([128, cap], f32, tag="uw")
                    nc.vector.tensor_mul(uw[:], ups[:], wbt[:])
                    nc.vector.tensor_mul(
                        ht[:, h * cap : (h + 1) * cap], sg[:], uw[:]
                    )

                stage = ysp.tile([128, C_CHUNKS * cap], f16, tag="yst")
                for cp in range(CT_PAIRS):
                    wdt = wdp.tile([128, 2 * HCH * 128], f16, tag="wd")
                    nc.sync.dma_start(wdt[:], wd.ap()[u, cp])
                    for k in range(2):
                        ct = 2 * cp + k
                        koff = k * HCH * 128
                        yps = psy.tile([128, cap], f32, tag="y")
                        for h in range(HCH):
                            nc.tensor.matmul(
                                yps[:],
                                wdt[:, koff + h * 128 : koff + (h + 1) * 128],
                                ht[:, h * cap : (h + 1) * cap],
                                start=(h == 0),
                                stop=(h == HCH - 1),
                            )
                        nc.vector.tensor_copy(
                            stage[:, ct * cap : (ct + 1) * cap], yps[:]
                        )
                half = (C_CHUNKS // 2) * cap
                nc.sync.dma_start(yts[u].ap()[:, :half], stage[:, :half])
                nc.sync.dma_start(yts[u].ap()[:, half:], stage[:, half:])
    nc.compile()
    return nc


def _get_nc(caps: tuple):
    if caps not in _NC_CACHE:
        _NC_CACHE[caps] = _build_nc(caps)
    return _NC_CACHE[caps]


def _route(xf: np.ndarray, gate_inp: np.ndarray):
    """Host gating in fp64: per-expert token index lists + combine weights."""
    logits = xf.astype(np.float64) @ gate_inp.astype(np.float64).T  # [N, E]
    # top-4 (descending); fp64 makes ordering robust vs the fp32 reference
    topi = np.argsort(-logits, axis=1, kind="stable")[:, :TOPK]  # [N, K]
    topv = np.take_along_axis(logits, topi, axis=1)
    w = np.exp(topv - topv[:, :1])
    w /= w.sum(axis=1, keepdims=True)  # [N, K] fp64 softmax
    idxs, wts = [], []
    for e in range(E):
        sel = topi == e  # [N, K]
        rows = np.nonzero(sel.any(axis=1))[0]
        k_of_row = np.argmax(sel[rows], axis=1)  # which top-k slot holds e
        idxs.append(rows.astype(np.int64))
        wts.append(w[rows, k_of_row])
    return idxs, wts


def _prepare(x, W_up, W_gate, W_down, gate_inp):
    xf = np.ascontiguousarray(np.asarray(x, dtype=np.float32)).reshape(N, C)
    W_up = np.asarray(W_up, dtype=np.float32)
    W_gate = np.asarray(W_gate, dtype=np.float32)
    W_down = np.asarray(W_down, dtype=np.float32)
    gate_inp = np.asarray(gate_inp, dtype=np.float32)

    idxs, wts = _route(xf, gate_inp)
    counts = [len(i) for i in idxs]

    # units: (expert, half), sorted desc by token count; rank blocks of 8
    units = sorted(
        ((e, hf) for e in range(E) for hf in range(HALVES)),
        key=lambda u: -counts[u[0]],
    )
    assign = [[units[j * N_CORES + core] for j in range(N_SLOTS)]
              for core in range(N_CORES)]
    caps = tuple(
        _pad4(max(counts[units[j * N_CORES + k][0]] for k in range(N_CORES)))
        for j in range(N_SLOTS)
    )

    # per-expert cached prep: gathered x (f16) and transposed weight forms
    xg_e = {}
    upt_e, gpt_e, wdt_e = {}, {}, {}
    for e in range(E):
        if counts[e]:
            xg_e[e] = xf[idxs[e]].astype(np.float16)
        # [h_chunk, q(c_in), c_chunk, h_col] -> [28, 128, 16*128]
        upt_e[e] = np.ascontiguousarray(
            W_up[e].reshape(H_CHUNKS, 128, C_CHUNKS, 128).transpose(0, 3, 2, 1)
        ).reshape(H_CHUNKS, 128, C_CHUNKS * 128).astype(np.float16)
        gpt_e[e] = np.ascontiguousarray(
            W_gate[e].reshape(H_CHUNKS, 128, C_CHUNKS, 128).transpose(0, 3, 2, 1)
        ).reshape(H_CHUNKS, 128, C_CHUNKS * 128).astype(np.float16)
        # [c_tile, q(h_in), h_chunk, c_col] -> [16, 128, 28, 128]
        wdt_e[e] = np.ascontiguousarray(
            W_down[e].reshape(C_CHUNKS, 128, H_CHUNKS, 128).transpose(0, 3, 2, 1)
        ).astype(np.float16)

    in_maps = []
    for core in range(N_CORES):
        wug = np.empty((N_SLOTS, HCH, 128, 2 * C_CHUNKS * 128), np.float16)
        wd = np.empty((N_SLOTS, CT_PAIRS, 128, 2 * HCH * 128), np.float16)
        im = {"wug": wug, "wd": wd}
        for j in range(N_SLOTS):
            cap = caps[j]
            e, hf = assign[core][j]
            h0 = hf * HCH
            idx, wvec = idxs[e], wts[e]
            cnt = len(idx)
            xge = np.zeros((cap, C), np.float16)
            if cnt:
                xge[:cnt] = xg_e[e]
            # [q, c_chunk, t] <- xge[t, c_chunk*128+q]
            im[f"xg{j}"] = np.ascontiguousarray(
                xge.reshape(cap, C_CHUNKS, 128).transpose(2, 1, 0)
            ).reshape(128, C_CHUNKS * cap)
            wb = np.zeros((128, cap), np.float32)
            wb[:, :cnt] = np.float32(wvec)[None, :]
            im[f"wb{j}"] = wb
            wug[j, :, :, : C_CHUNKS * 128] = upt_e[e][h0 : h0 + HCH]
            wug[j, :, :, C_CHUNKS * 128 :] = gpt_e[e][h0 : h0 + HCH]
            wdu = wdt_e[e][:, :, h0 : h0 + HCH, :].reshape(C_CHUNKS, 128, HCH * 128)
            wd[j, :, :, : HCH * 128] = wdu[0::2]
            wd[j, :, :, HCH * 128 :] = wdu[1::2]
        in_maps.append(im)
    return in_maps, caps, assign, idxs


def _combine(results, caps, assign, idxs):
    y = np.zeros((N, C), np.float32)
    for core in range(N_CORES):
        for j in range(N_SLOTS):
            e, _hf = assign[core][j]
            idx = idxs[e]
            cnt = len(idx)
            if not cnt:
                continue
            # yt [128, 16*cap]: value at (p, ct*cap + t) = y[token t, ct*128+p]
            ytf = (
                results[core][f"yt{j}"]
                .reshape(128, C_CHUNKS, caps[j])
                .transpose(1, 0, 2)
                .reshape(C, caps[j])
            )
            y[idx] += ytf[:, :cnt].T.astype(np.float32)
    return y.reshape(B, T, C)


def kernel(x, W_up, W_gate, W_down, gate_inp):
    from concourse import bass_utils

    in_maps, caps, assign, idxs = _prepare(x, W_up, W_gate, W_down, gate_inp)
    nc = _get_nc(caps)
    res = bass_utils.run_bass_kernel_spmd(nc, in_maps, core_ids=list(range(N_CORES)))
    kernel.last_result = res
    return _combine(res.results, caps, assign, idxs)
